# revision 1
# baseline (speedup 1.0000x reference)
"""MultiHeadEMABlock Trainium2 kernel (8-core SPMD, bass/Tile).

Math (reference):
  h = LayerNorm_c(x[b,c,n] over c) * gamma + beta          (per (b,n))
  xe[b,n,h,d] = h[b,n,d] * expansion[h,d]
  y = causal damped EMA along n: y[t] = a_h*sum_{s<=t} q_h^{t-s} xe[s]
  out[b,d,n] = sum_h y[b,n,h,d]*reduction[h,d] + x

Identities used:
  - Per-(h,d) scales commute with the EMA (it mixes along n only):
      out = x + sum_h rho_h[d] * S_h[d,n],  rho_h[d] = a_h*e[h,d]*r[h,d]*gamma[d]
      S_h = EMA(q_h) applied to the normalized input z.
  - beta contributes a data-independent low-rank term added on host (exact).

Sharding: 8 cores = 4 batches x 2 sequence halves. Each core processes its
half plus a W-column halo from the left (zero-padded for the first half);
q^W underflows, so results are exact without any cross-core collective.

Device algorithm (per core, c-major [channel x n] base layout):
  1. LayerNorm stats via replicated ones-matmuls on PE; z = (x-m)*rstd (DVE),
     rstd = exp(-0.5*ln(var+eps)) on ACT (Rsqrt table is unusable here).
  2. EMA as chunked causal convolution on PE, chunk L=128:
     - scale+transpose fused: one matmul per (chunk,dtile,headgroup) with a
       diag(rho_h) packed rhs turns c-major z into n-major per-head scaled
       inputs X_h (4 heads per N=512 matmul).
     - per chunk, 8 lower-triangular T_h matmuls head-accumulate in PSUM,
       plus a K=8 rank-8 carry-correction matmul (q_h^{i+1} profiles).
     - carries tracked per head via an unscaled transpose + end-row matmul
       (E), propagated with tiny [8,512] DVE ops.
  3. Back-transpose to c-major via identity matmuls, residual add on GpSimd,
     DMA out.
"""
import contextlib
import ctypes
import sys
import types

import numpy as np

for _p in ("/root/.axon_site/_ro/trn_rl_repo", "/opt/trn_rl_repo"):
    if _p not in sys.path:
        sys.path.append(_p)

B, C, N, H = 4, 512, 4096, 8
EPS = 1e-5
N_CORES = 8
NHALF = N // 2
CT = C // 128  # channel tiles
L = 128  # EMA chunk length


# ---------------------------------------------------------------------------
# axon NTFF shim (lets run_bass_kernel_spmd(trace=True) capture HW profiles)
# ---------------------------------------------------------------------------
def _install_ntff_shim():
    if "antenv.axon_hooks" in sys.modules:
        return
    holder = {"hook": None}

    def _make(so_path):
        try:
            lib = ctypes.CDLL(so_path)
        except OSError:
            return None
        if not hasattr(lib, "axon_start_nrt_profile"):
            return None
        lib.axon_start_nrt_profile.argtypes = [
            ctypes.POINTER(ctypes.c_int64),
            ctypes.c_size_t,
        ]
        lib.axon_start_nrt_profile.restype = ctypes.c_int64
        lib.axon_stop_nrt_profile.argtypes = [ctypes.c_char_p]
        lib.axon_stop_nrt_profile.restype = ctypes.c_int64

        @contextlib.contextmanager
        def _hook(output_dir, device_ids):
            import jax

            jax.devices()
            if device_ids:
                ids = (ctypes.c_int64 * len(device_ids))(*device_ids)
                rc = lib.axon_start_nrt_profile(ids, len(device_ids))
            else:
                rc = lib.axon_start_nrt_profile(None, 0)
            if rc != 0:
                raise RuntimeError(f"axon_start_nrt_profile rc={rc}")
            try:
                yield
            finally:
                n = lib.axon_stop_nrt_profile(str(output_dir).encode())
                print(f"ntff profile: {n} file(s) -> {output_dir}", file=sys.stderr)

        return _hook

    mod = types.ModuleType("antenv.axon_hooks")
    mod.set_axon_ntff_profile_hook = lambda h: holder.__setitem__("hook", h)
    mod.get_axon_ntff_profile_hook = lambda: holder["hook"]
    sys.modules["antenv.axon_hooks"] = mod
    try:
        import antenv

        antenv.axon_hooks = mod
    except ImportError:
        pass
    holder["hook"] = _make("/opt/axon/libaxon_pjrt.so")


def _split_multiwait(nc, max_waits=1):
    """This walrus build rejects >1 sync wait per instruction; split extras
    onto same-engine NoOps inserted just before (per-engine order is the
    execution order, so semantics are preserved)."""
    from concourse import mybir

    k = [0]
    for fn in nc.m.functions:
        for blk in fn.blocks:
            out = []
            for inst in blk.instructions:
                si = getattr(inst, "sync_info", None)
                if si is not None and len(si.on_wait) > max_waits:
                    waits = list(si.on_wait)
                    for w in waits[max_waits:]:
                        k[0] += 1
                        out.append(
                            mybir.InstNoOp(
                                name=f"{inst.name}-mw{k[0]}",
                                sync_info=mybir.SyncInfo(on_wait=[w], on_update=[]),
                                bass_nofuse=True,
                                engine=inst.engine,
                            )
                        )
                    inst.sync_info = mybir.SyncInfo(
                        on_wait=waits[:max_waits], on_update=list(si.on_update)
                    )
                out.append(inst)
            blk.instructions[:] = out


# ---------------------------------------------------------------------------
# program builder
# ---------------------------------------------------------------------------
def build_program(W):
    """Build the SPMD per-core program. W: halo width (multiple of L)."""
    import concourse.bass as bass
    import concourse.tile as tile
    from concourse import mybir

    NW = NHALF + W
    K0 = W // L
    NCH = NW // L  # chunks
    # ragged 512-wide stat chunks
    stat_slices = []
    o = 0
    while o < NW:
        w = min(512, NW - o)
        stat_slices.append((o, w))
        o += w
    f32 = mybir.dt.float32
    bf16 = mybir.dt.bfloat16
    Op = mybir.AluOpType
    Act = mybir.ActivationFunctionType

    nc = bass.Bass(
        "TRN2",
        target_bir_lowering=False,
        debug=False,
        enable_asserts=False,
        num_devices=N_CORES,
    )
    xs_d = nc.dram_tensor("xs", [C, NW], f32, kind="ExternalInput").ap()
    tm_d = nc.dram_tensor("tmats", [H * 128, 128], bf16, kind="ExternalInput").ap()
    w4_d = nc.dram_tensor("w4", [H * 128, 512], bf16, kind="ExternalInput").ap()
    ek_d = nc.dram_tensor("ek", [128, H], bf16, kind="ExternalInput").ap()
    pm_d = nc.dram_tensor("pmat", [H, 128], bf16, kind="ExternalInput").ap()
    id_d = nc.dram_tensor("ident", [128, 128], bf16, kind="ExternalInput").ap()
    rh_d = nc.dram_tensor("rho_hd", [H, C], f32, kind="ExternalInput").ap()
    ql_d = nc.dram_tensor("qlcol", [H, 1], f32, kind="ExternalInput").ap()
    out_d = nc.dram_tensor("out_t", [C, NHALF], f32, kind="ExternalOutput").ap()

    with tile.TileContext(nc) as tc:
        with contextlib.ExitStack() as ctx:
            pers = ctx.enter_context(tc.tile_pool(name="pers", bufs=1))
            xs_pool = ctx.enter_context(tc.tile_pool(name="xsp", bufs=2))
            sq_pool = ctx.enter_context(tc.tile_pool(name="sqp", bufs=4))
            ps_pool = ctx.enter_context(tc.tile_pool(name="ps", bufs=1, space="PSUM"))
            st_pool = ctx.enter_context(tc.tile_pool(name="stats", bufs=3))
            xh_pool = ctx.enter_context(tc.tile_pool(name="xhp", bufs=4))
            xu_pool = ctx.enter_context(tc.tile_pool(name="xup", bufs=4))
            cr_pool = ctx.enter_context(tc.tile_pool(name="crp", bufs=3))
            s_pool = ctx.enter_context(tc.tile_pool(name="sp", bufs=4))
            out_pool = ctx.enter_context(tc.tile_pool(name="outp", bufs=4))

            # ---- small constants (sync queue, cheap) ----
            ek = pers.tile([128, H], bf16, tag="ek")
            nc.sync.dma_start(out=ek[:], in_=ek_d)
            pmat = pers.tile([H, 128], bf16, tag="pmat")
            nc.sync.dma_start(out=pmat[:], in_=pm_d)
            ident = pers.tile([128, 128], bf16, tag="ident")
            nc.sync.dma_start(out=ident[:], in_=id_d)
            rho = pers.tile([H, C], f32, tag="rho")
            nc.sync.dma_start(out=rho[:], in_=rh_d)
            qlc = pers.tile([H, 1], f32, tag="qlc")
            nc.sync.dma_start(out=qlc[:], in_=ql_d)
            epsb = pers.tile([128, 1], f32, tag="eps")
            nc.gpsimd.memset(epsb[:], EPS)
            ones = pers.tile([128, 128], bf16, tag="ones")
            nc.gpsimd.memset(ones[:], 1.0 / C)
            # big constants on the scalar HWDGE queue so they don't delay xs
            T8 = [pers.tile([128, 128], bf16, tag=f"T{h}", name=f"T{h}") for h in range(H)]
            for h in range(H):
                nc.scalar.dma_start(out=T8[h][:], in_=tm_d[h * 128 : (h + 1) * 128, :])
            W4 = [pers.tile([128, 512], bf16, tag=f"W4_{i}", name=f"W4_{i}") for i in range(H)]
            for i in range(H):
                nc.scalar.dma_start(out=W4[i][:], in_=w4_d[i * 128 : (i + 1) * 128, :])

            # ---- load, cast, square (per stat-chunk pieces for fast ramp) ----
            xb = pers.tile([128, CT * NW], bf16, tag="xb")
            z = pers.tile([128, CT * NW], bf16, tag="z")
            xsq = [pers.tile([128, NW], bf16, tag=f"sq{ct}", name=f"sq{ct}")
                   for ct in range(CT)]
            for o, wd in stat_slices:
                for ct in range(CT):
                    xst = xs_pool.tile([128, 512], f32, tag="xs", bufs=6)
                    nc.sync.dma_start(
                        out=xst[:, :wd],
                        in_=xs_d[ct * 128 : (ct + 1) * 128, o : o + wd],
                    )
                    nc.vector.tensor_scalar(
                        out=xb[:, ct * NW + o : ct * NW + o + wd], in0=xst[:, :wd],
                        scalar1=1.0, scalar2=None, op0=Op.mult,
                    )
                    nc.scalar.square(out=xsq[ct][:, o : o + wd], in_=xst[:, :wd])

            # ---- layernorm stats + z ----
            for o, wd in stat_slices:
                ps_m = ps_pool.tile([128, 512], f32, tag="ema", bufs=2)
                ps_s = ps_pool.tile([128, 512], f32, tag="ema", bufs=2)
                for ct in range(CT):
                    nc.tensor.matmul(
                        out=ps_m[:, :wd], lhsT=ones[:],
                        rhs=xb[:, ct * NW + o : ct * NW + o + wd],
                        start=(ct == 0), stop=(ct == CT - 1),
                    )
                for ct in range(CT):
                    nc.tensor.matmul(
                        out=ps_s[:, :wd], lhsT=ones[:], rhs=xsq[ct][:, o : o + wd],
                        start=(ct == 0), stop=(ct == CT - 1),
                    )
                mean_bf = st_pool.tile([128, 512], bf16, tag="meanbf")
                nc.scalar.activation(out=mean_bf[:, :wd], in_=ps_m[:, :wd], func=Act.Copy)
                m2 = st_pool.tile([128, 512], f32, tag="m2")
                nc.scalar.square(out=m2[:, :wd], in_=ps_m[:, :wd])
                var = st_pool.tile([128, 512], f32, tag="var")
                nc.vector.scalar_tensor_tensor(
                    out=var[:, :wd], in0=ps_s[:, :wd], scalar=0.0, in1=m2[:, :wd],
                    op0=Op.bypass, op1=Op.subtract,
                )
                lnv = st_pool.tile([128, 512], f32, tag="lnv")
                nc.scalar.activation(out=lnv[:, :wd], in_=var[:, :wd], func=Act.Ln, bias=epsb[:])
                rstd = st_pool.tile([128, 512], bf16, tag="rstd")
                nc.scalar.activation(out=rstd[:, :wd], in_=lnv[:, :wd], func=Act.Exp, scale=-0.5)
                for ct in range(CT):
                    t = st_pool.tile([128, 512], bf16, tag="tnorm")
                    nc.vector.tensor_tensor(
                        out=t[:, :wd], in0=xb[:, ct * NW + o : ct * NW + o + wd],
                        in1=mean_bf[:, :wd], op=Op.subtract,
                    )
                    nc.vector.tensor_tensor(
                        out=z[:, ct * NW + o : ct * NW + o + wd], in0=t[:, :wd],
                        in1=rstd[:, :wd], op=Op.mult,
                    )

            # ---- EMA chunks ----
            c_cur = cr_pool.tile([H, C], f32, tag="carry")
            nc.gpsimd.memset(c_cur[:], 0.0)

            def z_slice(k, dt):
                return z[:, dt * NW + k * L : dt * NW + (k + 1) * L]

            def carry_end(k):
                """X_u transpose + end-row matmul E_k; returns e_ps."""
                xu_ps = ps_pool.tile([128, 512], f32, tag="misc", bufs=2)
                for dt in range(CT):
                    nc.tensor.matmul(
                        out=xu_ps[:, dt * 128 : (dt + 1) * 128],
                        lhsT=z_slice(k, dt), rhs=ident[:], start=True, stop=True,
                    )
                xu = xu_pool.tile([128, 512], bf16, tag="xu")
                nc.scalar.activation(out=xu[:], in_=xu_ps[:], func=Act.Copy)
                e_ps = ps_pool.tile([H, 512], f32, tag="misc", bufs=2)
                nc.tensor.matmul(out=e_ps[:], lhsT=ek[:], rhs=xu[:], start=True,
                                 stop=True)
                return e_ps

            def carry_update(c_prev, e_ps):
                c_nxt = cr_pool.tile([H, C], f32, tag="carry")
                c_tmp = cr_pool.tile([H, C], f32, tag="ctmp")
                nc.vector.tensor_scalar(
                    out=c_tmp[:], in0=c_prev[:], scalar1=qlc[:, 0:1], scalar2=None,
                    op0=Op.mult,
                )
                nc.vector.tensor_tensor(out=c_nxt[:], in0=c_tmp[:], in1=e_ps[:],
                                        op=Op.add)
                return c_nxt

            def make_xh(k):
                """scaled transposes: xh cols = g*2048 + dt*512 + h'*128 + jj"""
                xh = xh_pool.tile([128, H * 512], bf16, tag="xh")
                for g in range(2):
                    for dp in range(2):
                        sp = ps_pool.tile([128, 1024], f32, tag="xps", bufs=2)
                        for dd in range(2):
                            dt = dp * 2 + dd
                            nc.tensor.matmul(
                                out=sp[:, dd * 512 : (dd + 1) * 512],
                                lhsT=z_slice(k, dt), rhs=W4[g * CT + dt][:],
                                start=True, stop=True,
                            )
                        dst = xh[:, g * 2048 + dp * 1024 : g * 2048 + (dp + 1) * 1024]
                        if (g + dp) % 2 == 0:
                            nc.scalar.activation(out=dst, in_=sp[:], func=Act.Copy)
                        else:
                            nc.vector.tensor_scalar(
                                out=dst, in0=sp[:], scalar1=1.0, scalar2=None,
                                op0=Op.mult,
                            )
                return xh[:].rearrange("p (g dt hp jj) -> p g dt hp jj",
                                       g=2, dt=CT, hp=4)

            def make_crho(c):
                c_rho = cr_pool.tile([H, C], bf16, tag="crho")
                nc.vector.tensor_tensor(out=c_rho[:], in0=c[:], in1=rho[:], op=Op.mult)
                return c_rho

            def chunk_tail(k, ema_ps):
                """back-transpose (PE identity matmuls) + residual + store"""
                s_sb = s_pool.tile([128, 512], bf16, tag="ssb")
                nc.scalar.activation(out=s_sb[:], in_=ema_ps[:], func=Act.Copy)
                t_ps = ps_pool.tile([128, 512], f32, tag="misc", bufs=2)
                for dt in range(CT):
                    nc.tensor.matmul(
                        out=t_ps[:, dt * 128 : (dt + 1) * 128],
                        lhsT=s_sb[:, dt * 128 : (dt + 1) * 128], rhs=ident[:],
                        start=True, stop=True,
                    )
                o_sb = s_pool.tile([128, 512], bf16, tag="osb")
                nc.scalar.activation(out=o_sb[:], in_=t_ps[:], func=Act.Copy)
                ot = out_pool.tile([128, 512], f32, tag="out")
                resid = xb.rearrange("p (dt t) -> p dt t", dt=CT)[
                    :, :, k * L : (k + 1) * L
                ]
                nc.gpsimd.tensor_tensor(
                    out=ot[:].rearrange("p (dt i) -> p dt i", dt=CT),
                    in0=o_sb[:].rearrange("p (dt i) -> p dt i", dt=CT),
                    in1=resid, op=Op.add,
                )
                ko = k - K0
                nc.sync.dma_start(
                    out=out_d.rearrange("(dt p) n -> p dt n", dt=CT)[
                        :, :, ko * L : (ko + 1) * L
                    ],
                    in_=ot[:].rearrange("p (dt i) -> p dt i", dt=CT),
                )

            for k in range(K0):  # halo chunks: carries only
                e_ps = carry_end(k)
                c_cur = carry_update(c_cur, e_ps)

            ks = list(range(K0, NCH))
            pairs = [ks[i : i + 2] for i in range(0, len(ks), 2)]
            for pair in pairs:
                xhs, crhos, psums = [], [], []
                for k in pair:
                    last = k == NCH - 1
                    e_ps = None if last else carry_end(k)
                    xhs.append(make_xh(k))
                    crhos.append(make_crho(c_cur))
                    if not last:
                        c_cur = carry_update(c_cur, e_ps)
                for h in range(H):  # interleave pair to reuse T8[h] stationary
                    g, hp = divmod(h, 4)
                    for i, k in enumerate(pair):
                        if h == 0:
                            psums.append(ps_pool.tile([128, 512], f32, tag="ema",
                                                      bufs=2, name=f"emaps{k}"))
                        nc.tensor.matmul(
                            out=psums[i][:], lhsT=T8[h][:], rhs=xhs[i][:, g, :, hp, :],
                            start=(h == 0), stop=False,
                        )
                for i, k in enumerate(pair):
                    nc.tensor.matmul(
                        out=psums[i][:], lhsT=pmat[:], rhs=crhos[i][:], start=False,
                        stop=True,
                    )
                for i, k in enumerate(pair):
                    chunk_tail(k, psums[i])
    return nc


def _host_params(ln_gamma, ln_beta, expansion, reduction, alphas, dampen_factors):
    import ml_dtypes

    a = 1.0 / (1.0 + np.exp(-alphas.astype(np.float64)))
    q = (1.0 - a) / (1.0 + np.exp(-dampen_factors.astype(np.float64)))
    qmax = float(q.max())
    W = L
    while qmax**W > 1e-12 and W < NHALF:
        W += L
    rho = (
        a[:, None]
        * expansion.astype(np.float64)
        * reduction.astype(np.float64)
        * ln_gamma.astype(np.float64)[None, :]
    )  # [H, C]
    bf = ml_dtypes.bfloat16
    ii, jj = np.meshgrid(np.arange(L), np.arange(L), indexing="ij")
    tmats = np.zeros((H * 128, 128), bf)
    for h in range(H):
        M = np.where(ii >= jj, q[h] ** np.maximum(ii - jj, 0), 0.0)  # T_h[i,j]
        tmats[h * 128 : (h + 1) * 128, :] = M.T.astype(bf)  # lhsT[j,i]
    w4 = np.zeros((H * 128, 512), bf)
    for g in range(2):
        for dt in range(CT):
            blk = np.zeros((128, 512))
            for hp in range(4):
                h = g * 4 + hp
                blk[:, hp * 128 : (hp + 1) * 128] = np.diag(rho[h, dt * 128 : (dt + 1) * 128])
            w4[(g * CT + dt) * 128 : (g * CT + dt + 1) * 128, :] = blk.astype(bf)
    ek = np.zeros((128, H), bf)
    for h in range(H):
        ek[:, h] = (q[h] ** (L - 1 - np.arange(L))).astype(bf)
    pmat = np.zeros((H, 128), bf)
    for h in range(H):
        pmat[h, :] = (q[h] ** (np.arange(L) + 1.0)).astype(bf)
    ident = np.eye(128, dtype=bf)
    rho_hd = rho.astype(np.float32)
    qlcol = (q**L).astype(np.float32).reshape(H, 1)
    consts = dict(
        tmats=tmats, w4=w4, ek=ek, pmat=pmat, ident=ident, rho_hd=rho_hd,
        qlcol=qlcol,
    )
    return a, q, W, consts


def _beta_term(ln_beta, expansion, reduction, a, q):
    if not np.any(ln_beta):
        return None
    n_idx = np.arange(N, dtype=np.float64)
    Cn = a[:, None] * (1.0 - q[:, None] ** (n_idx[None, :] + 1.0)) / (1.0 - q[:, None])
    w = (
        expansion.astype(np.float64)
        * reduction.astype(np.float64)
        * ln_beta.astype(np.float64)[None, :]
    )
    return np.einsum("hc,hn->cn", w, Cn).astype(np.float32)


def _make_in_maps(x, W, consts):
    NW = NHALF + W
    in_maps = []
    for core in range(N_CORES):
        b, half = divmod(core, 2)
        xs = np.zeros((C, NW), np.float32)
        s = half * NHALF - W
        if s < 0:
            xs[:, W:] = x[b, :, :NHALF]
        else:
            xs[:] = x[b, :, s : s + NW]
        in_maps.append(dict(consts, xs=xs))
    return in_maps


def kernel(x, ln_gamma, ln_beta, expansion, reduction, alphas, dampen_factors,
           trace=False):
    _install_ntff_shim()
    from concourse.bass_utils import run_bass_kernel_spmd
    from concourse.bass_interp import get_hw_module

    x = np.asarray(x, np.float32)
    a, q, W, consts = _host_params(
        np.asarray(ln_gamma), np.asarray(ln_beta), np.asarray(expansion),
        np.asarray(reduction), np.asarray(alphas), np.asarray(dampen_factors),
    )
    nc = build_program(W)
    _split_multiwait(nc)
    nc.m = get_hw_module(nc.m)

    in_maps = _make_in_maps(x, W, consts)
    res = run_bass_kernel_spmd(
        nc, in_maps, core_ids=list(range(N_CORES)), trace=trace
    )

    out = np.empty((B, C, N), np.float32)
    for core in range(N_CORES):
        b, half = divmod(core, 2)
        out[b, :, half * NHALF : (half + 1) * NHALF] = res.results[core]["out_t"]
    bt = _beta_term(
        np.asarray(ln_beta), np.asarray(expansion), np.asarray(reduction), a, q
    )
    if bt is not None:
        out += bt[None]
    if trace:
        kernel.last_results = res
    return out



# revision 3
# speedup vs baseline: 1.0611x; 1.0611x over previous
"""MultiHeadEMABlock Trainium2 kernel (8-core SPMD, bass/Tile) — v2.

Math (reference):
  h = LayerNorm_c(x[b,c,n] over c) * gamma + beta          (per (b,n))
  xe[b,n,h,d] = h[b,n,d] * expansion[h,d]
  y = causal damped EMA along n: y[t] = a_h*sum_{s<=t} q_h^{t-s} xe[s]
  out[b,d,n] = sum_h y[b,n,h,d]*reduction[h,d] + x

Identities used:
  - Per-(h,d) scales commute with the EMA (it mixes along n only):
      out = x + sum_h rho_h[d] * S_h[d,n],  rho_h[d] = a_h*e[h,d]*r[h,d]*gamma[d]
      S_h = EMA(q_h) applied to the normalized input z.
  - beta contributes a data-independent low-rank term added on host (exact).
  - rstd is position-wise, so it commutes with the c->n transpose: apply it
    as a per-partition scale while evacuating the transposed PSUM.

Sharding: 8 cores = 4 batches x 2 sequence halves, W=128 left halo
(q_max^128 << 1e-8 so no cross-core collective).

Device algorithm (per core, c-major [channel x n] base layout):
  1. x loaded via SWDGE cast-DMA (f32->bf16). Mean via ones-matmul
     (replicated); zc = xb - mean on GpSimd. Position-column stats
     (mean,sumsq) via tiny N=1 matmuls; rstd column r_col = exp(-.5 ln(var))
     on ACT over [128, nchunk] tiles. Stats groups are emitted interleaved
     with the chunk pairs that consume them.
  2. EMA as chunked causal convolution on PE, chunk L=128:
     - per dtile one LDW serves three matmuls: two scale+transpose (4-head
       diag rhs) and one identity transpose (carry end-rows); PSUM evacuated
       with the per-partition r_col scale fused in (DVE tensor_scalar / ACT
       activation-scale).
     - 8 lower-triangular T_h matmuls head-accumulate in PSUM + rank-8
       carry-correction matmul; carry update = one fused
       scalar_tensor_tensor; c*rho on GpSimd.
  3. Back-transpose to c-major; residual add fused into the PSUM evacuation
     (DVE tensor_tensor); bf16 out DMA per chunk pair, host casts f32.
"""
import contextlib
import ctypes
import sys
import types

import numpy as np

for _p in ("/root/.axon_site/_ro/trn_rl_repo", "/opt/trn_rl_repo"):
    if _p not in sys.path:
        sys.path.append(_p)

B, C, N, H = 4, 512, 4096, 8
EPS = 1e-5
N_CORES = 8
NHALF = N // 2
CT = C // 128  # channel tiles
L = 128  # EMA chunk length
W = 128  # halo (q_max^128 < 1e-30 for this problem; assert at host)
NW = NHALF + W
K0 = W // L
NCH = NW // L
GSZ = 4  # chunks per stat group
OUT_BF16 = True  # device emits bf16 output; host casts to f32


# ---------------------------------------------------------------------------
# axon NTFF shim (lets run_bass_kernel_spmd(trace=True) capture HW profiles)
# ---------------------------------------------------------------------------
def _install_ntff_shim():
    if "antenv.axon_hooks" in sys.modules:
        return
    holder = {"hook": None}

    def _make(so_path):
        try:
            lib = ctypes.CDLL(so_path)
        except OSError:
            return None
        if not hasattr(lib, "axon_start_nrt_profile"):
            return None
        lib.axon_start_nrt_profile.argtypes = [
            ctypes.POINTER(ctypes.c_int64),
            ctypes.c_size_t,
        ]
        lib.axon_start_nrt_profile.restype = ctypes.c_int64
        lib.axon_stop_nrt_profile.argtypes = [ctypes.c_char_p]
        lib.axon_stop_nrt_profile.restype = ctypes.c_int64

        @contextlib.contextmanager
        def _hook(output_dir, device_ids):
            import jax

            jax.devices()
            if device_ids:
                ids = (ctypes.c_int64 * len(device_ids))(*device_ids)
                rc = lib.axon_start_nrt_profile(ids, len(device_ids))
            else:
                rc = lib.axon_start_nrt_profile(None, 0)
            if rc != 0:
                raise RuntimeError(f"axon_start_nrt_profile rc={rc}")
            try:
                yield
            finally:
                n = lib.axon_stop_nrt_profile(str(output_dir).encode())
                print(f"ntff profile: {n} file(s) -> {output_dir}", file=sys.stderr)

        return _hook

    mod = types.ModuleType("antenv.axon_hooks")
    mod.set_axon_ntff_profile_hook = lambda h: holder.__setitem__("hook", h)
    mod.get_axon_ntff_profile_hook = lambda: holder["hook"]
    sys.modules["antenv.axon_hooks"] = mod
    try:
        import antenv

        antenv.axon_hooks = mod
    except ImportError:
        pass
    holder["hook"] = _make("/opt/axon/libaxon_pjrt.so")


def _split_multiwait(nc, max_waits=1):
    """This walrus build rejects >1 sync wait per instruction; split extras
    onto same-engine NoOps inserted just before (per-engine order is the
    execution order, so semantics are preserved)."""
    from concourse import mybir

    k = [0]
    for fn in nc.m.functions:
        for blk in fn.blocks:
            out = []
            for inst in blk.instructions:
                si = getattr(inst, "sync_info", None)
                if si is not None and len(si.on_wait) > max_waits:
                    waits = list(si.on_wait)
                    for w in waits[max_waits:]:
                        k[0] += 1
                        out.append(
                            mybir.InstNoOp(
                                name=f"{inst.name}-mw{k[0]}",
                                sync_info=mybir.SyncInfo(on_wait=[w], on_update=[]),
                                bass_nofuse=True,
                                engine=inst.engine,
                            )
                        )
                    inst.sync_info = mybir.SyncInfo(
                        on_wait=waits[:max_waits], on_update=list(si.on_update)
                    )
                out.append(inst)
            blk.instructions[:] = out


# ---------------------------------------------------------------------------
# program builder
# ---------------------------------------------------------------------------
def build_program():
    import concourse.bass as bass
    import concourse.tile as tile
    from concourse import mybir

    # ragged 512-wide stat groups; chunk k belongs to group k // GSZ
    stat_slices = []
    o = 0
    while o < NW:
        w = min(GSZ * L, NW - o)
        stat_slices.append((o, w))
        o += w
    f32 = mybir.dt.float32
    bf16 = mybir.dt.bfloat16
    out_dt = bf16 if OUT_BF16 else f32
    Op = mybir.AluOpType
    Act = mybir.ActivationFunctionType

    nc = bass.Bass(
        "TRN2",
        target_bir_lowering=False,
        debug=False,
        enable_asserts=False,
        num_devices=N_CORES,
    )
    xs_d = nc.dram_tensor("xs", [C, NW], f32, kind="ExternalInput").ap()
    tm_d = nc.dram_tensor("tmats", [H * 128, 128], bf16, kind="ExternalInput").ap()
    w4_d = nc.dram_tensor("w4", [H * 128, 512], bf16, kind="ExternalInput").ap()
    ek_d = nc.dram_tensor("ek", [128, H], bf16, kind="ExternalInput").ap()
    pm_d = nc.dram_tensor("pmat", [H, 128], bf16, kind="ExternalInput").ap()
    id_d = nc.dram_tensor("ident", [128, 128], bf16, kind="ExternalInput").ap()
    rh_d = nc.dram_tensor("rho_hd", [H, C], f32, kind="ExternalInput").ap()
    ql_d = nc.dram_tensor("qlcol", [H, 1], f32, kind="ExternalInput").ap()
    oc_d = nc.dram_tensor("onecol", [128, 2], bf16, kind="ExternalInput").ap()
    out_d = nc.dram_tensor("out_t", [C, NHALF], out_dt, kind="ExternalOutput").ap()

    with tile.TileContext(nc) as tc:
        with contextlib.ExitStack() as ctx:
            pers = ctx.enter_context(tc.tile_pool(name="pers", bufs=1))
            sq_pool = ctx.enter_context(tc.tile_pool(name="sqp", bufs=2))
            ps_pool = ctx.enter_context(tc.tile_pool(name="ps", bufs=1, space="PSUM"))
            st_pool = ctx.enter_context(tc.tile_pool(name="stats", bufs=2))
            xh_pool = ctx.enter_context(tc.tile_pool(name="xhp", bufs=2))
            xu_pool = ctx.enter_context(tc.tile_pool(name="xup", bufs=3))
            cr_pool = ctx.enter_context(tc.tile_pool(name="crp", bufs=3))
            s_pool = ctx.enter_context(tc.tile_pool(name="sp", bufs=3))
            out_pool = ctx.enter_context(tc.tile_pool(name="outp", bufs=3))
            rc_pool = ctx.enter_context(tc.tile_pool(name="rcp", bufs=3))

            # ---- small constants (sync queue, cheap) ----
            ek = pers.tile([128, H], bf16, tag="ek")
            nc.sync.dma_start(out=ek[:], in_=ek_d)
            pmat = pers.tile([H, 128], bf16, tag="pmat")
            nc.sync.dma_start(out=pmat[:], in_=pm_d)
            ident = pers.tile([128, 128], bf16, tag="ident")
            nc.sync.dma_start(out=ident[:], in_=id_d)
            rho = pers.tile([H, C], f32, tag="rho")
            nc.sync.dma_start(out=rho[:], in_=rh_d)
            qlc = pers.tile([H, 1], f32, tag="qlc")
            nc.sync.dma_start(out=qlc[:], in_=ql_d)
            onecol = pers.tile([128, 2], bf16, tag="onecol")
            nc.sync.dma_start(out=onecol[:], in_=oc_d)
            epsb = pers.tile([128, 1], f32, tag="eps")
            nc.gpsimd.memset(epsb[:], EPS)
            ones = pers.tile([128, 128], bf16, tag="ones")
            nc.gpsimd.memset(ones[:], 1.0 / C)
            # big constants on the scalar HWDGE queue so they don't delay xs
            T8 = [pers.tile([128, 128], bf16, tag=f"T{h}", name=f"T{h}") for h in range(H)]
            for h in range(H):
                nc.scalar.dma_start(out=T8[h][:], in_=tm_d[h * 128 : (h + 1) * 128, :])
            W4 = [pers.tile([128, 512], bf16, tag=f"W4_{i}", name=f"W4_{i}") for i in range(H)]
            for i in range(H):
                nc.scalar.dma_start(out=W4[i][:], in_=w4_d[i * 128 : (i + 1) * 128, :])

            # ---- loads: one SWDGE cast-DMA (f32->bf16) per stat group ----
            xb = pers.tile([128, CT * NW], bf16, tag="xb")
            zc = pers.tile([128, CT * NW], bf16, tag="zc")
            xs3 = xs_d.rearrange("(ct p) n -> p ct n", ct=CT)
            xb3 = xb[:].rearrange("p (ct n) -> p ct n", ct=CT)
            zc3 = zc[:].rearrange("p (ct n) -> p ct n", ct=CT)
            for o, wd in stat_slices:
                nc.gpsimd.dma_start(
                    out=xb3[:, :, o : o + wd], in_=xs3[:, :, o : o + wd]
                )

            rcols = {}  # group -> [128, nk] f32 rstd columns

            def emit_stats(g):
                o, wd = stat_slices[g]
                nk = wd // L
                ps_m = ps_pool.tile([128, 512], f32, tag="misc", bufs=2)
                for ct in range(CT):
                    nc.tensor.matmul(
                        out=ps_m[:, :wd], lhsT=ones[:], rhs=xb3[:, ct, o : o + wd],
                        start=(ct == 0), stop=(ct == CT - 1),
                    )
                m_rep = st_pool.tile([128, 512], bf16, tag="meanbf")
                nc.scalar.activation(out=m_rep[:, :wd], in_=ps_m[:, :wd], func=Act.Copy)
                # zc = xb - mean (GpSimd), squares (DVE/GpSimd split)
                xsq = sq_pool.tile([128, CT * 512], bf16, tag="xsq", name=f"xsq{g}")
                xsq3 = xsq[:].rearrange("p (ct n) -> p ct n", ct=CT)
                for ct in range(CT):
                    nc.gpsimd.tensor_tensor(
                        out=zc3[:, ct, o : o + wd], in0=xb3[:, ct, o : o + wd],
                        in1=m_rep[:, :wd], op=Op.subtract,
                    )
                    eng = nc.vector if ct % 2 == 0 else nc.gpsimd
                    eng.tensor_tensor(
                        out=xsq3[:, ct, :wd], in0=xb3[:, ct, o : o + wd],
                        in1=xb3[:, ct, o : o + wd], op=Op.mult,
                    )
                # position-column stats via tiny matmuls
                scol_ps = ps_pool.tile([128, 2 * nk], f32, tag="misc", bufs=2)
                for kk in range(nk):
                    nc.tensor.matmul(
                        out=scol_ps[:, 2 * kk : 2 * kk + 1],
                        lhsT=m_rep[:, kk * L : (kk + 1) * L],
                        rhs=onecol[:, 0:1], start=True, stop=True,
                    )
                    for ct in range(CT):
                        nc.tensor.matmul(
                            out=scol_ps[:, 2 * kk + 1 : 2 * kk + 2],
                            lhsT=xsq3[:, ct, kk * L : (kk + 1) * L],
                            rhs=onecol[:, 1:2],
                            start=(ct == 0), stop=(ct == CT - 1),
                        )
                scol = st_pool.tile([128, 2 * nk], f32, tag="scol")
                nc.vector.tensor_scalar(
                    out=scol[:], in0=scol_ps[:], scalar1=1.0, scalar2=None,
                    op0=Op.mult,
                )
                sc3 = scol[:].rearrange("p (k two) -> p k two", two=2)
                m2 = st_pool.tile([128, nk], f32, tag="m2c")
                nc.scalar.square(out=m2[:], in_=sc3[:, :, 0])
                var = st_pool.tile([128, nk], f32, tag="varc")
                nc.vector.scalar_tensor_tensor(
                    out=var[:], in0=sc3[:, :, 1], scalar=0.0,
                    in1=m2[:], op0=Op.bypass, op1=Op.subtract,
                )
                lnv = st_pool.tile([128, nk], f32, tag="lnvc")
                nc.scalar.activation(out=lnv[:], in_=var[:], func=Act.Ln, bias=epsb[:])
                rc = rc_pool.tile([128, nk], f32, tag="rcol", name=f"rcol{g}")
                nc.scalar.activation(out=rc[:], in_=lnv[:], func=Act.Exp, scale=-0.5)
                rcols[g] = rc

            def r_col(k):
                g, kk = divmod(k, GSZ)
                return rcols[g][:, kk : kk + 1]

            def zc_slice(k, dt):
                return zc3[:, dt, k * L : (k + 1) * L]

            def carry_update(c_prev, e_ps):
                c_nxt = cr_pool.tile([H, C], f32, tag="carry")
                nc.vector.scalar_tensor_tensor(
                    out=c_nxt[:], in0=c_prev[:], scalar=qlc[:, 0:1], in1=e_ps[:],
                    op0=Op.mult, op1=Op.add,
                )
                return c_nxt

            def make_xh(k, want_e):
                """scaled transposes + (optional) carry end-rows, one LDW per
                dtile. xh cols = g*2048 + dp*1024 + dd*512 + hp*128."""
                xh = xh_pool.tile([128, H * 512], bf16, tag="xh")
                xu_ps = None
                if want_e:
                    xu_ps = ps_pool.tile([128, 512], f32, tag="misc", bufs=2)
                for dp in range(2):
                    sps = [ps_pool.tile([128, 1024], f32, tag="xps", bufs=2,
                                        name=f"xps{k}_{dp}_{g}") for g in range(2)]
                    for dd in range(2):
                        dt = dp * 2 + dd
                        for g in range(2):
                            nc.tensor.matmul(
                                out=sps[g][:, dd * 512 : (dd + 1) * 512],
                                lhsT=zc_slice(k, dt), rhs=W4[g * CT + dt][:],
                                start=True, stop=True,
                            )
                        if want_e:
                            nc.tensor.matmul(
                                out=xu_ps[:, dt * 128 : (dt + 1) * 128],
                                lhsT=zc_slice(k, dt), rhs=ident[:],
                                start=True, stop=True,
                            )
                    for g in range(2):
                        dst = xh[:, g * 2048 + dp * 1024 : g * 2048 + (dp + 1) * 1024]
                        if (g + dp) % 2 == 0:
                            nc.vector.tensor_scalar(
                                out=dst, in0=sps[g][:], scalar1=r_col(k),
                                scalar2=None, op0=Op.mult,
                            )
                        else:
                            nc.scalar.activation(
                                out=dst, in_=sps[g][:], func=Act.Copy, scale=r_col(k)
                            )
                e_ps = None
                if want_e:
                    xu = xu_pool.tile([128, 512], bf16, tag="xu")
                    nc.vector.tensor_scalar(
                        out=xu[:], in0=xu_ps[:], scalar1=r_col(k), scalar2=None,
                        op0=Op.mult,
                    )
                    e_ps = ps_pool.tile([H, 512], f32, tag="misc", bufs=2)
                    nc.tensor.matmul(out=e_ps[:], lhsT=ek[:], rhs=xu[:],
                                     start=True, stop=True)
                return xh[:].rearrange("p (g dt hp jj) -> p g dt hp jj",
                                       g=2, dt=CT, hp=4), e_ps

            def halo_carry(k):
                xu_ps = ps_pool.tile([128, 512], f32, tag="misc", bufs=2)
                for dt in range(CT):
                    nc.tensor.matmul(
                        out=xu_ps[:, dt * 128 : (dt + 1) * 128],
                        lhsT=zc_slice(k, dt), rhs=ident[:], start=True, stop=True,
                    )
                xu = xu_pool.tile([128, 512], bf16, tag="xu")
                nc.vector.tensor_scalar(
                    out=xu[:], in0=xu_ps[:], scalar1=r_col(k), scalar2=None,
                    op0=Op.mult,
                )
                e_ps = ps_pool.tile([H, 512], f32, tag="misc", bufs=2)
                nc.tensor.matmul(out=e_ps[:], lhsT=ek[:], rhs=xu[:], start=True,
                                 stop=True)
                return e_ps

            def make_crho(c):
                c_rho = cr_pool.tile([H, C], bf16, tag="crho")
                nc.gpsimd.tensor_tensor(out=c_rho[:], in0=c[:], in1=rho[:], op=Op.mult)
                return c_rho

            def chunk_tail(k, ema_ps, ot, half):
                """back-transposes + fused residual evac into pair tile"""
                s_sb = s_pool.tile([128, 512], bf16, tag="ssb")
                nc.scalar.activation(out=s_sb[:], in_=ema_ps[:], func=Act.Copy)
                t_ps = ps_pool.tile([128, 512], f32, tag="ema", bufs=2)
                for dt in range(CT):
                    nc.tensor.matmul(
                        out=t_ps[:, dt * 128 : (dt + 1) * 128],
                        lhsT=s_sb[:, dt * 128 : (dt + 1) * 128], rhs=ident[:],
                        start=True, stop=True,
                    )
                resid = xb3[:, :, k * L : (k + 1) * L]
                ot3 = ot[:].rearrange("p (dt i) -> p dt i", dt=CT)
                nc.vector.tensor_tensor(
                    out=ot3[:, :, half * L : (half + 1) * L],
                    in0=t_ps[:].rearrange("p (dt i) -> p dt i", dt=CT),
                    in1=resid, op=Op.add,
                )

            # ---- emission: stats groups interleaved with chunk pairs ----
            c_cur = cr_pool.tile([H, C], f32, tag="carry")
            nc.gpsimd.memset(c_cur[:], 0.0)

            ks = list(range(K0, NCH))
            pairs = [ks[i : i + 2] for i in range(0, len(ks), 2)]
            emitted = set()

            def need_group(g):
                if g not in emitted and g < len(stat_slices):
                    emitted.add(g)
                    emit_stats(g)

            need_group(0)
            for k in range(K0):  # halo chunks: carries only
                e_ps = halo_carry(k)
                c_cur = carry_update(c_cur, e_ps)

            for pair in pairs:
                for k in pair:
                    need_group(k // GSZ)
                    need_group((k + 2) // GSZ)  # prefetch stats for next pair
                xhs, crhos, psums = [], [], []
                for k in pair:
                    last = k == NCH - 1
                    xh, e_ps = make_xh(k, want_e=not last)
                    xhs.append(xh)
                    crhos.append(make_crho(c_cur))
                    if not last:
                        c_cur = carry_update(c_cur, e_ps)
                for h in range(H):  # interleave pair to reuse T8[h] stationary
                    g, hp = divmod(h, 4)
                    for i, k in enumerate(pair):
                        if h == 0:
                            psums.append(ps_pool.tile([128, 512], f32, tag="ema",
                                                      bufs=2, name=f"emaps{k}"))
                        nc.tensor.matmul(
                            out=psums[i][:], lhsT=T8[h][:], rhs=xhs[i][:, g, :, hp, :],
                            start=(h == 0), stop=False,
                        )
                for i, k in enumerate(pair):
                    nc.tensor.matmul(
                        out=psums[i][:], lhsT=pmat[:], rhs=crhos[i][:], start=False,
                        stop=True,
                    )
                ot = out_pool.tile([128, CT * 2 * L], out_dt, tag="out")
                for i, k in enumerate(pair):
                    chunk_tail(k, psums[i], ot, i)
                ko = pair[0] - K0
                nc.sync.dma_start(
                    out=out_d.rearrange("(dt p) n -> p dt n", dt=CT)[
                        :, :, ko * L : (ko + 2) * L
                    ],
                    in_=ot[:].rearrange("p (dt i) -> p dt i", dt=CT),
                )
    return nc


def _host_params(ln_gamma, ln_beta, expansion, reduction, alphas, dampen_factors):
    import ml_dtypes

    a = 1.0 / (1.0 + np.exp(-alphas.astype(np.float64)))
    q = (1.0 - a) / (1.0 + np.exp(-dampen_factors.astype(np.float64)))
    qmax = float(q.max())
    assert qmax**W < 1e-8, f"halo W={W} too small for qmax={qmax}"
    rho = (
        a[:, None]
        * expansion.astype(np.float64)
        * reduction.astype(np.float64)
        * ln_gamma.astype(np.float64)[None, :]
    )  # [H, C]
    bf = ml_dtypes.bfloat16
    ii, jj = np.meshgrid(np.arange(L), np.arange(L), indexing="ij")
    tmats = np.zeros((H * 128, 128), bf)
    for h in range(H):
        M = np.where(ii >= jj, q[h] ** np.maximum(ii - jj, 0), 0.0)  # T_h[i,j]
        tmats[h * 128 : (h + 1) * 128, :] = M.T.astype(bf)  # lhsT[j,i]
    w4 = np.zeros((H * 128, 512), bf)
    for g in range(2):
        for dt in range(CT):
            blk = np.zeros((128, 512))
            for hp in range(4):
                h = g * 4 + hp
                blk[:, hp * 128 : (hp + 1) * 128] = np.diag(rho[h, dt * 128 : (dt + 1) * 128])
            w4[(g * CT + dt) * 128 : (g * CT + dt + 1) * 128, :] = blk.astype(bf)
    ek = np.zeros((128, H), bf)
    for h in range(H):
        ek[:, h] = (q[h] ** (L - 1 - np.arange(L))).astype(bf)
    pmat = np.zeros((H, 128), bf)
    for h in range(H):
        pmat[h, :] = (q[h] ** (np.arange(L) + 1.0)).astype(bf)
    ident = np.eye(128, dtype=bf)
    rho_hd = rho.astype(np.float32)
    qlcol = (q**L).astype(np.float32).reshape(H, 1)
    onecol = np.zeros((128, 2), bf)
    onecol[:, 0] = 1.0 / 128.0
    onecol[:, 1] = 1.0 / C
    consts = dict(
        tmats=tmats, w4=w4, ek=ek, pmat=pmat, ident=ident, rho_hd=rho_hd,
        qlcol=qlcol, onecol=onecol,
    )
    return a, q, consts


def _beta_term(ln_beta, expansion, reduction, a, q):
    if not np.any(ln_beta):
        return None
    n_idx = np.arange(N, dtype=np.float64)
    Cn = a[:, None] * (1.0 - q[:, None] ** (n_idx[None, :] + 1.0)) / (1.0 - q[:, None])
    w = (
        expansion.astype(np.float64)
        * reduction.astype(np.float64)
        * ln_beta.astype(np.float64)[None, :]
    )
    return np.einsum("hc,hn->cn", w, Cn).astype(np.float32)


def _make_in_maps(x, consts):
    in_maps = []
    for core in range(N_CORES):
        b, half = divmod(core, 2)
        xs = np.zeros((C, NW), np.float32)
        s = half * NHALF - W
        if s < 0:
            xs[:, W:] = x[b, :, :NHALF]
        else:
            xs[:] = x[b, :, s : s + NW]
        in_maps.append(dict(consts, xs=xs))
    return in_maps


def kernel(x, ln_gamma, ln_beta, expansion, reduction, alphas, dampen_factors,
           trace=False):
    _install_ntff_shim()
    from concourse.bass_utils import run_bass_kernel_spmd
    from concourse.bass_interp import get_hw_module

    x = np.asarray(x, np.float32)
    a, q, consts = _host_params(
        np.asarray(ln_gamma), np.asarray(ln_beta), np.asarray(expansion),
        np.asarray(reduction), np.asarray(alphas), np.asarray(dampen_factors),
    )
    nc = build_program()
    _split_multiwait(nc)
    nc.m = get_hw_module(nc.m)

    in_maps = _make_in_maps(x, consts)
    res = run_bass_kernel_spmd(
        nc, in_maps, core_ids=list(range(N_CORES)), trace=trace
    )

    out = np.empty((B, C, N), np.float32)
    for core in range(N_CORES):
        b, half = divmod(core, 2)
        out[b, :, half * NHALF : (half + 1) * NHALF] = np.asarray(
            res.results[core]["out_t"], np.float32
        )
    bt = _beta_term(
        np.asarray(ln_beta), np.asarray(expansion), np.asarray(reduction), a, q
    )
    if bt is not None:
        out += bt[None]
    if trace:
        kernel.last_results = res
    return out


# revision 4
# speedup vs baseline: 1.3277x; 1.2513x over previous
"""MultiHeadEMABlock Trainium2 kernel (8-core SPMD, bass/Tile) — v3.

Math (reference):
  h = LayerNorm_c(x[b,c,n] over c) * gamma + beta          (per (b,n))
  xe[b,n,h,d] = h[b,n,d] * expansion[h,d]
  y = causal damped EMA along n: y[t] = a_h*sum_{s<=t} q_h^{t-s} xe[s]
  out[b,d,n] = sum_h y[b,n,h,d]*reduction[h,d] + x

Identities used:
  - Per-(h,d) scales commute with the EMA: out = x + sum_h rho_h[d]*S_h[d,n].
  - RANK-5 BASIS: the 8 exponential kernels a_h q_h^l (l in [0,160)) lie in a
    rank-5 subspace (SVD, max per-head rel err 1.9e-4). With basis phi_b and
    per-channel coefficients c_b[d] = sum_h rho_h[d] beta[h,b], the head sum
    collapses to 5 "basis heads":
      out_ema[d,t] = sum_b c_b[d] * (phi_b (*) z)[d,t]
  - q_max^128 ~ 1e-31, so each 128-chunk needs only its own + the previous
    chunk as history: cross-chunk carry state is replaced by a second
    triangular matmul (PHI2) against the PREVIOUS chunk's transposed inputs.
    No serial carry chain at all.
  - rstd is position-wise so it commutes with the c->n transpose: applied as
    a per-partition scale while evacuating the transposed PSUM.
  - beta(LN) contributes a data-independent term added on host (exact).

Sharding: 8 cores = 4 batches x 2 sequence halves, W=128 left halo.

Device algorithm (per core, c-major [channel x n] base layout):
  1. x loaded via SWDGE cast-DMA (f32->bf16). Mean via ones-matmul
     (replicated); zc = xb - mean on GpSimd. Position-column stats via tiny
     N=1 matmuls; r_col = exp(-.5 ln(var+eps)) on ACT over [128, nk] tiles.
  2. Per chunk: one LDW per dtile serves two scale+transpose matmuls (basis
     0-3 diag rhs N=512, basis-4 plane N=128); PSUM evacuated with the
     per-partition r_col scale fused (DVE tensor_scalar / ACT act-scale).
     Then per basis: T5 matmul (this chunk) + PHI2 matmul (previous chunk)
     head-accumulate in PSUM, pair-interleaved for stationary reuse.
  3. Back-transpose to c-major; residual add fused into the PSUM evacuation
     (DVE tensor_tensor); bf16 out DMA per chunk pair, host casts f32.
"""
import contextlib
import ctypes
import sys
import types

import numpy as np

for _p in ("/root/.axon_site/_ro/trn_rl_repo", "/opt/trn_rl_repo"):
    if _p not in sys.path:
        sys.path.append(_p)

B, C, N, H = 4, 512, 4096, 8
EPS = 1e-5
N_CORES = 8
NHALF = N // 2
CT = C // 128  # channel tiles
L = 128  # EMA chunk length
W = 128  # halo (q_max^128 < 1e-30 for this problem; assert at host)
NW = NHALF + W
K0 = W // L
NCH = NW // L
GSZ = 4  # chunks per stat group
R = 5  # basis rank
LAGS = 160
OUT_BF16 = True  # device emits bf16 output; host casts to f32


# ---------------------------------------------------------------------------
# axon NTFF shim (lets run_bass_kernel_spmd(trace=True) capture HW profiles)
# ---------------------------------------------------------------------------
def _install_ntff_shim():
    if "antenv.axon_hooks" in sys.modules:
        return
    holder = {"hook": None}

    def _make(so_path):
        try:
            lib = ctypes.CDLL(so_path)
        except OSError:
            return None
        if not hasattr(lib, "axon_start_nrt_profile"):
            return None
        lib.axon_start_nrt_profile.argtypes = [
            ctypes.POINTER(ctypes.c_int64),
            ctypes.c_size_t,
        ]
        lib.axon_start_nrt_profile.restype = ctypes.c_int64
        lib.axon_stop_nrt_profile.argtypes = [ctypes.c_char_p]
        lib.axon_stop_nrt_profile.restype = ctypes.c_int64

        @contextlib.contextmanager
        def _hook(output_dir, device_ids):
            import jax

            jax.devices()
            if device_ids:
                ids = (ctypes.c_int64 * len(device_ids))(*device_ids)
                rc = lib.axon_start_nrt_profile(ids, len(device_ids))
            else:
                rc = lib.axon_start_nrt_profile(None, 0)
            if rc != 0:
                raise RuntimeError(f"axon_start_nrt_profile rc={rc}")
            try:
                yield
            finally:
                n = lib.axon_stop_nrt_profile(str(output_dir).encode())
                print(f"ntff profile: {n} file(s) -> {output_dir}", file=sys.stderr)

        return _hook

    mod = types.ModuleType("antenv.axon_hooks")
    mod.set_axon_ntff_profile_hook = lambda h: holder.__setitem__("hook", h)
    mod.get_axon_ntff_profile_hook = lambda: holder["hook"]
    sys.modules["antenv.axon_hooks"] = mod
    try:
        import antenv

        antenv.axon_hooks = mod
    except ImportError:
        pass
    holder["hook"] = _make("/opt/axon/libaxon_pjrt.so")


def _split_multiwait(nc, max_waits=1):
    """This walrus build rejects >1 sync wait per instruction; split extras
    onto same-engine NoOps inserted just before (per-engine order is the
    execution order, so semantics are preserved)."""
    from concourse import mybir

    k = [0]
    for fn in nc.m.functions:
        for blk in fn.blocks:
            out = []
            for inst in blk.instructions:
                si = getattr(inst, "sync_info", None)
                if si is not None and len(si.on_wait) > max_waits:
                    waits = list(si.on_wait)
                    for w in waits[max_waits:]:
                        k[0] += 1
                        out.append(
                            mybir.InstNoOp(
                                name=f"{inst.name}-mw{k[0]}",
                                sync_info=mybir.SyncInfo(on_wait=[w], on_update=[]),
                                bass_nofuse=True,
                                engine=inst.engine,
                            )
                        )
                    inst.sync_info = mybir.SyncInfo(
                        on_wait=waits[:max_waits], on_update=list(si.on_update)
                    )
                out.append(inst)
            blk.instructions[:] = out


# ---------------------------------------------------------------------------
# program builder
# ---------------------------------------------------------------------------
def build_program():
    import concourse.bass as bass
    import concourse.tile as tile
    from concourse import mybir

    stat_slices = []
    o = 0
    while o < NW:
        w = min(GSZ * L, NW - o)
        stat_slices.append((o, w))
        o += w
    f32 = mybir.dt.float32
    bf16 = mybir.dt.bfloat16
    out_dt = bf16 if OUT_BF16 else f32
    Op = mybir.AluOpType
    Act = mybir.ActivationFunctionType

    nc = bass.Bass(
        "TRN2",
        target_bir_lowering=False,
        debug=False,
        enable_asserts=False,
        num_devices=N_CORES,
    )
    xs_d = nc.dram_tensor("xs", [C, NW], f32, kind="ExternalInput").ap()
    t5_d = nc.dram_tensor("t5", [R * 128, 128], bf16, kind="ExternalInput").ap()
    p2_d = nc.dram_tensor("phi2", [R * 128, 128], bf16, kind="ExternalInput").ap()
    w5_d = nc.dram_tensor("w5", [CT * 128, 512], bf16, kind="ExternalInput").ap()
    w5p_d = nc.dram_tensor("w5p", [CT * 128, 128], bf16, kind="ExternalInput").ap()
    id_d = nc.dram_tensor("ident", [128, 128], bf16, kind="ExternalInput").ap()
    oc_d = nc.dram_tensor("onecol", [128, 2], bf16, kind="ExternalInput").ap()
    out_d = nc.dram_tensor("out_t", [C, NHALF], out_dt, kind="ExternalOutput").ap()

    with tile.TileContext(nc) as tc:
        with contextlib.ExitStack() as ctx:
            pers = ctx.enter_context(tc.tile_pool(name="pers", bufs=1))
            sq_pool = ctx.enter_context(tc.tile_pool(name="sqp", bufs=2))
            ps_pool = ctx.enter_context(tc.tile_pool(name="ps", bufs=1, space="PSUM"))
            st_pool = ctx.enter_context(tc.tile_pool(name="stats", bufs=2))
            xh_pool = ctx.enter_context(tc.tile_pool(name="xhp", bufs=3))
            s_pool = ctx.enter_context(tc.tile_pool(name="sp", bufs=3))
            out_pool = ctx.enter_context(tc.tile_pool(name="outp", bufs=3))
            rc_pool = ctx.enter_context(tc.tile_pool(name="rcp", bufs=3))

            # ---- small constants (sync queue, cheap) ----
            ident = pers.tile([128, 128], bf16, tag="ident")
            nc.sync.dma_start(out=ident[:], in_=id_d)
            onecol = pers.tile([128, 2], bf16, tag="onecol")
            nc.sync.dma_start(out=onecol[:], in_=oc_d)
            epsb = pers.tile([128, 1], f32, tag="eps")
            nc.gpsimd.memset(epsb[:], EPS)
            ones = pers.tile([128, 128], bf16, tag="ones")
            nc.gpsimd.memset(ones[:], 1.0 / C)
            # big constants on the scalar HWDGE queue so they don't delay xs
            T5 = [pers.tile([128, 128], bf16, tag=f"T{b}", name=f"T{b}") for b in range(R)]
            for b in range(R):
                nc.scalar.dma_start(out=T5[b][:], in_=t5_d[b * 128 : (b + 1) * 128, :])
            P2 = [pers.tile([128, 128], bf16, tag=f"P{b}", name=f"P{b}") for b in range(R)]
            for b in range(R):
                nc.scalar.dma_start(out=P2[b][:], in_=p2_d[b * 128 : (b + 1) * 128, :])
            W5 = [pers.tile([128, 512], bf16, tag=f"W5_{i}", name=f"W5_{i}") for i in range(CT)]
            for i in range(CT):
                nc.scalar.dma_start(out=W5[i][:], in_=w5_d[i * 128 : (i + 1) * 128, :])
            W5P = [pers.tile([128, 128], bf16, tag=f"W5P_{i}", name=f"W5P_{i}") for i in range(CT)]
            for i in range(CT):
                nc.scalar.dma_start(out=W5P[i][:], in_=w5p_d[i * 128 : (i + 1) * 128, :])

            # ---- loads: one SWDGE cast-DMA (f32->bf16) per stat group ----
            xb = pers.tile([128, CT * NW], bf16, tag="xb")
            zc = pers.tile([128, CT * NW], bf16, tag="zc")
            xs3 = xs_d.rearrange("(ct p) n -> p ct n", ct=CT)
            xb3 = xb[:].rearrange("p (ct n) -> p ct n", ct=CT)
            zc3 = zc[:].rearrange("p (ct n) -> p ct n", ct=CT)
            for o, wd in stat_slices:
                nc.gpsimd.dma_start(
                    out=xb3[:, :, o : o + wd], in_=xs3[:, :, o : o + wd]
                )

            rcols = {}

            def emit_stats(g):
                o, wd = stat_slices[g]
                nk = wd // L
                ps_m = ps_pool.tile([128, 512], f32, tag="misc", bufs=2)
                for ct in range(CT):
                    nc.tensor.matmul(
                        out=ps_m[:, :wd], lhsT=ones[:], rhs=xb3[:, ct, o : o + wd],
                        start=(ct == 0), stop=(ct == CT - 1),
                    )
                m_rep = st_pool.tile([128, 512], bf16, tag="meanbf")
                nc.scalar.activation(out=m_rep[:, :wd], in_=ps_m[:, :wd], func=Act.Copy)
                xsq = sq_pool.tile([128, CT * 512], bf16, tag="xsq", name=f"xsq{g}")
                xsq3 = xsq[:].rearrange("p (ct n) -> p ct n", ct=CT)
                for ct in range(CT):
                    nc.gpsimd.tensor_tensor(
                        out=zc3[:, ct, o : o + wd], in0=xb3[:, ct, o : o + wd],
                        in1=m_rep[:, :wd], op=Op.subtract,
                    )
                    eng = nc.vector if ct % 2 == 0 else nc.gpsimd
                    eng.tensor_tensor(
                        out=xsq3[:, ct, :wd], in0=xb3[:, ct, o : o + wd],
                        in1=xb3[:, ct, o : o + wd], op=Op.mult,
                    )
                scol_ps = ps_pool.tile([128, 2 * nk], f32, tag="misc", bufs=2)
                for kk in range(nk):
                    nc.tensor.matmul(
                        out=scol_ps[:, 2 * kk : 2 * kk + 1],
                        lhsT=m_rep[:, kk * L : (kk + 1) * L],
                        rhs=onecol[:, 0:1], start=True, stop=True,
                    )
                    for ct in range(CT):
                        nc.tensor.matmul(
                            out=scol_ps[:, 2 * kk + 1 : 2 * kk + 2],
                            lhsT=xsq3[:, ct, kk * L : (kk + 1) * L],
                            rhs=onecol[:, 1:2],
                            start=(ct == 0), stop=(ct == CT - 1),
                        )
                scol = st_pool.tile([128, 2 * nk], f32, tag="scol")
                nc.vector.tensor_scalar(
                    out=scol[:], in0=scol_ps[:], scalar1=1.0, scalar2=None,
                    op0=Op.mult,
                )
                sc3 = scol[:].rearrange("p (k two) -> p k two", two=2)
                m2 = st_pool.tile([128, nk], f32, tag="m2c")
                nc.scalar.square(out=m2[:], in_=sc3[:, :, 0])
                var = st_pool.tile([128, nk], f32, tag="varc")
                nc.vector.scalar_tensor_tensor(
                    out=var[:], in0=sc3[:, :, 1], scalar=0.0,
                    in1=m2[:], op0=Op.bypass, op1=Op.subtract,
                )
                lnv = st_pool.tile([128, nk], f32, tag="lnvc")
                nc.scalar.activation(out=lnv[:], in_=var[:], func=Act.Ln, bias=epsb[:])
                rc = rc_pool.tile([128, nk], f32, tag="rcol", name=f"rcol{g}")
                nc.scalar.activation(out=rc[:], in_=lnv[:], func=Act.Exp, scale=-0.5)
                rcols[g] = rc

            def r_col(k):
                g, kk = divmod(k, GSZ)
                return rcols[g][:, kk : kk + 1]

            def zc_slice(k, dt):
                return zc3[:, dt, k * L : (k + 1) * L]

            def make_xh(k):
                """scaled transposes: xh cols = dt*512 + b*128 + c (basis b<4)
                plus basis-4 plane xp4 cols = dt*128 + c. One LDW per dtile."""
                xh = xh_pool.tile([128, 4 * 512], bf16, tag="xh")
                xp4 = xh_pool.tile([128, 512], bf16, tag="xp4")
                pl_ps = ps_pool.tile([128, 512], f32, tag="misc", bufs=2)
                for dp in range(2):
                    sp = ps_pool.tile([128, 1024], f32, tag="xps", bufs=2,
                                      name=f"xps{k}_{dp}")
                    for dd in range(2):
                        dt = dp * 2 + dd
                        nc.tensor.matmul(
                            out=sp[:, dd * 512 : (dd + 1) * 512],
                            lhsT=zc_slice(k, dt), rhs=W5[dt][:],
                            start=True, stop=True,
                        )
                        nc.tensor.matmul(
                            out=pl_ps[:, dt * 128 : (dt + 1) * 128],
                            lhsT=zc_slice(k, dt), rhs=W5P[dt][:],
                            start=True, stop=True,
                        )
                    dst = xh[:, dp * 1024 : (dp + 1) * 1024]
                    if dp == 0:
                        nc.vector.tensor_scalar(
                            out=dst, in0=sp[:], scalar1=r_col(k), scalar2=None,
                            op0=Op.mult,
                        )
                    else:
                        nc.scalar.activation(
                            out=dst, in_=sp[:], func=Act.Copy, scale=r_col(k)
                        )
                if k % 2 == 0:
                    nc.vector.tensor_scalar(
                        out=xp4[:], in0=pl_ps[:], scalar1=r_col(k), scalar2=None,
                        op0=Op.mult,
                    )
                else:
                    nc.scalar.activation(
                        out=xp4[:], in_=pl_ps[:], func=Act.Copy, scale=r_col(k)
                    )
                return (
                    xh[:].rearrange("p (dt b c) -> p dt b c", dt=CT, b=4),
                    xp4,
                )

            def rhs_b(xh_pair, b):
                xh4, xp4 = xh_pair
                if b < 4:
                    return xh4[:, :, b, :]
                return xp4[:]

            def chunk_tail(k, ema_ps, ot, half):
                s_sb = s_pool.tile([128, 512], bf16, tag="ssb")
                nc.scalar.activation(out=s_sb[:], in_=ema_ps[:], func=Act.Copy)
                t_ps = ps_pool.tile([128, 512], f32, tag="ema", bufs=2)
                for dt in range(CT):
                    nc.tensor.matmul(
                        out=t_ps[:, dt * 128 : (dt + 1) * 128],
                        lhsT=s_sb[:, dt * 128 : (dt + 1) * 128], rhs=ident[:],
                        start=True, stop=True,
                    )
                resid = xb3[:, :, k * L : (k + 1) * L]
                ot3 = ot[:].rearrange("p (dt i) -> p dt i", dt=CT)
                nc.vector.tensor_tensor(
                    out=ot3[:, :, half * L : (half + 1) * L],
                    in0=t_ps[:].rearrange("p (dt i) -> p dt i", dt=CT),
                    in1=resid, op=Op.add,
                )

            # ---- emission: stats groups interleaved with chunk pairs ----
            ks = list(range(K0, NCH))
            pairs = [ks[i : i + 2] for i in range(0, len(ks), 2)]
            emitted = set()

            def need_group(g):
                if g not in emitted and g < len(stat_slices):
                    emitted.add(g)
                    emit_stats(g)

            need_group(0)
            prev = None
            for k in range(K0):  # halo chunks: correction source only
                prev = make_xh(k)

            for pair in pairs:
                for k in pair:
                    need_group(k // GSZ)
                    need_group((k + 2) // GSZ)  # prefetch stats for next pair
                xhs, psums = [], []
                for k in pair:
                    xhs.append(make_xh(k))
                prevs = [prev, xhs[0]]
                for b in range(R):  # this-chunk triangular, pair-interleaved
                    for i, k in enumerate(pair):
                        if b == 0:
                            psums.append(ps_pool.tile([128, 512], f32, tag="ema",
                                                      bufs=2, name=f"emaps{k}"))
                        nc.tensor.matmul(
                            out=psums[i][:], lhsT=T5[b][:], rhs=rhs_b(xhs[i], b),
                            start=(b == 0), stop=False,
                        )
                for b in range(R):  # previous-chunk correction
                    for i, k in enumerate(pair):
                        nc.tensor.matmul(
                            out=psums[i][:], lhsT=P2[b][:], rhs=rhs_b(prevs[i], b),
                            start=False, stop=(b == R - 1),
                        )
                prev = xhs[-1]
                ot = out_pool.tile([128, CT * 2 * L], out_dt, tag="out")
                for i, k in enumerate(pair):
                    chunk_tail(k, psums[i], ot, i)
                ko = pair[0] - K0
                nc.sync.dma_start(
                    out=out_d.rearrange("(dt p) n -> p dt n", dt=CT)[
                        :, :, ko * L : (ko + 2) * L
                    ],
                    in_=ot[:].rearrange("p (dt i) -> p dt i", dt=CT),
                )
    return nc


def _host_params(ln_gamma, ln_beta, expansion, reduction, alphas, dampen_factors):
    import ml_dtypes

    a = 1.0 / (1.0 + np.exp(-alphas.astype(np.float64)))
    q = (1.0 - a) / (1.0 + np.exp(-dampen_factors.astype(np.float64)))
    qmax = float(q.max())
    assert qmax**W < 1e-8, f"halo W={W} too small for qmax={qmax}"
    rho = (  # WITHOUT a_h: amplitude lives in the kernel matrix M
        expansion.astype(np.float64)
        * reduction.astype(np.float64)
        * ln_gamma.astype(np.float64)[None, :]
    )  # [H, C]
    lag = np.arange(LAGS)
    M = a[:, None] * (q[:, None] ** lag[None, :])  # [H, LAGS]
    U, S, Vt = np.linalg.svd(M, full_matrices=False)
    beta = U[:, :R] * S[:R]  # [H, R]
    phi = Vt[:R]  # [R, LAGS]
    cb = np.einsum("hd,hb->bd", rho, beta)  # [R, C]

    bf = ml_dtypes.bfloat16
    ii, jj = np.meshgrid(np.arange(L), np.arange(L), indexing="ij")
    t5 = np.zeros((R * 128, 128), bf)
    p2 = np.zeros((R * 128, 128), bf)
    for b in range(R):
        lagm = ii - jj
        Tb = np.where(lagm >= 0, phi[b][np.clip(lagm, 0, LAGS - 1)], 0.0)
        t5[b * 128 : (b + 1) * 128, :] = Tb.T.astype(bf)  # lhsT[j,i]
        lag2 = ii + L - jj  # lag from previous chunk, in [1, 255]
        P2b = np.where(lag2 < LAGS, phi[b][np.clip(lag2, 0, LAGS - 1)], 0.0)
        p2[b * 128 : (b + 1) * 128, :] = P2b.T.astype(bf)
    w5 = np.zeros((CT * 128, 512), bf)
    w5p = np.zeros((CT * 128, 128), bf)
    for dt in range(CT):
        blk = np.zeros((128, 512))
        for b in range(4):
            blk[:, b * 128 : (b + 1) * 128] = np.diag(cb[b, dt * 128 : (dt + 1) * 128])
        w5[dt * 128 : (dt + 1) * 128, :] = blk.astype(bf)
        w5p[dt * 128 : (dt + 1) * 128, :] = np.diag(
            cb[4, dt * 128 : (dt + 1) * 128]
        ).astype(bf)
    ident = np.eye(128, dtype=bf)
    onecol = np.zeros((128, 2), bf)
    onecol[:, 0] = 1.0 / 128.0
    onecol[:, 1] = 1.0 / C
    consts = dict(t5=t5, phi2=p2, w5=w5, w5p=w5p, ident=ident, onecol=onecol)
    return a, q, consts


def _beta_term(ln_beta, expansion, reduction, a, q):
    if not np.any(ln_beta):
        return None
    n_idx = np.arange(N, dtype=np.float64)
    Cn = a[:, None] * (1.0 - q[:, None] ** (n_idx[None, :] + 1.0)) / (1.0 - q[:, None])
    w = (
        expansion.astype(np.float64)
        * reduction.astype(np.float64)
        * ln_beta.astype(np.float64)[None, :]
    )
    return np.einsum("hc,hn->cn", w, Cn).astype(np.float32)


def _make_in_maps(x, consts):
    in_maps = []
    for core in range(N_CORES):
        b, half = divmod(core, 2)
        xs = np.zeros((C, NW), np.float32)
        s = half * NHALF - W
        if s < 0:
            xs[:, W:] = x[b, :, :NHALF]
        else:
            xs[:] = x[b, :, s : s + NW]
        in_maps.append(dict(consts, xs=xs))
    return in_maps


def kernel(x, ln_gamma, ln_beta, expansion, reduction, alphas, dampen_factors,
           trace=False):
    _install_ntff_shim()
    from concourse.bass_utils import run_bass_kernel_spmd
    from concourse.bass_interp import get_hw_module

    x = np.asarray(x, np.float32)
    a, q, consts = _host_params(
        np.asarray(ln_gamma), np.asarray(ln_beta), np.asarray(expansion),
        np.asarray(reduction), np.asarray(alphas), np.asarray(dampen_factors),
    )
    nc = build_program()
    _split_multiwait(nc)
    nc.m = get_hw_module(nc.m)

    in_maps = _make_in_maps(x, consts)
    res = run_bass_kernel_spmd(
        nc, in_maps, core_ids=list(range(N_CORES)), trace=trace
    )

    out = np.empty((B, C, N), np.float32)
    for core in range(N_CORES):
        b, half = divmod(core, 2)
        out[b, :, half * NHALF : (half + 1) * NHALF] = np.asarray(
            res.results[core]["out_t"], np.float32
        )
    bt = _beta_term(
        np.asarray(ln_beta), np.asarray(expansion), np.asarray(reduction), a, q
    )
    if bt is not None:
        out += bt[None]
    if trace:
        kernel.last_results = res
    return out
